# revision 15
# baseline (speedup 1.0000x reference)
"""GCN layer on 8 Trainium2 NeuronCores.

  H = X @ W^T + b            (dense projection, node-sharded)
  out[r] = sum_e val[e] * H[col[e]]  over edges with row[e] == r

Strategy (per the dest-row sharding hint):
  - Launch 1 (SPMD x8): each core computes H for its 12500-node slice
    (bf16), returns it; host concatenates the shards (the "all-gather").
  - Launch 2 (SPMD x8): each core owns out rows [12500c, 12500(c+1)).
    Its edges are bucketed by source col range (4 buckets of 25000 so
    gather indices fit int16), sorted by dest row, and packed into
    fixed-size cells of CC*128 edge slots per (window, bucket), where a
    window is 128 consecutive out rows.  For each 128-edge chunk the
    device gathers H[col] rows via dma_gather (tokens land one per
    partition), builds a selector S[e, r] = val[e] * (iota[r] ==
    row_rel[e]) with one DVE tensor_scalar, and accumulates
    S^T @ msgs = [128 rows x 128 feat] into the window's PSUM tile via
    the TensorEngine.  Windows flush to HBM sequentially.

All host work is index/layout preprocessing; the projection, gather,
message scaling and segment reduction all run on the NeuronCores.
"""

import os
import sys

sys.path.insert(0, "/opt/trn_rl_repo")

import numpy as np
import ml_dtypes

import concourse.bacc as bacc
import concourse.mybir as mybir
from concourse import tile
from concourse.bass_utils import run_bass_kernel_spmd

BF16 = ml_dtypes.bfloat16

N_NODES = 100000
N_CORES = 8
R = N_NODES // N_CORES          # 12500 out rows per core
NW = (R + 127) // 128           # 98 windows of 128 rows
RPAD = NW * 128                 # 12544
IN_DIM = 256
F = 128                         # out features
NB = 4                          # col buckets
BUCKET = N_NODES // NB          # 25000 (< 32767 so int16 gather idx works)
NCELL = NB * NW                 # cells per core (bucket-major, then window)

_cache = {}


XB = 8  # windows per X/H DMA batch in launch 1


def _build_launch1():
    # h output is in "tiled" layout [128, NW*F]: row w*128+p of H lives at
    # h[p, w*F:(w+1)*F].  The host un-tiles it (free).
    nc = bacc.Bacc(None, target_bir_lowering=False)
    xt_p = nc.declare_dram_parameter("xt", [IN_DIM, RPAD], mybir.dt.bfloat16, isOutput=False)
    wt_p = nc.declare_dram_parameter("wt", [IN_DIM, F], mybir.dt.bfloat16, isOutput=False)
    b_p = nc.declare_dram_parameter("bb", [128, F], mybir.dt.float32, isOutput=False)
    h_p = nc.declare_dram_parameter("h", [128, NW * F], mybir.dt.bfloat16, isOutput=True)

    with tile.TileContext(nc) as tc:
        with (
            tc.tile_pool(name="const", bufs=1) as cpool,
            tc.tile_pool(name="x", bufs=3) as xpool,
            tc.tile_pool(name="h", bufs=3) as hpool,
            tc.tile_pool(name="ps", bufs=4, space="PSUM") as ppool,
        ):
            w_t = cpool.tile([128, 2 * F], mybir.dt.bfloat16)
            nc.sync.dma_start(out=w_t[:, 0:F], in_=wt_p[0:128, :])
            nc.sync.dma_start(out=w_t[:, F:2 * F], in_=wt_p[128:256, :])
            b_t = cpool.tile([128, F], mybir.dt.float32)
            nc.sync.dma_start(out=b_t[:], in_=b_p[:])

            for g0 in range(0, NW, XB):
                g1 = min(g0 + XB, NW)
                nwin = g1 - g0
                x_t = xpool.tile([128, 2 * XB * 128], mybir.dt.bfloat16)
                nc.sync.dma_start(
                    out=x_t[:, 0:nwin * 128],
                    in_=xt_p[0:128, g0 * 128:g1 * 128])
                nc.sync.dma_start(
                    out=x_t[:, XB * 128:XB * 128 + nwin * 128],
                    in_=xt_p[128:256, g0 * 128:g1 * 128])
                h_t = hpool.tile([128, XB * F], mybir.dt.bfloat16)
                for m in range(g0, g1):
                    o = (m - g0) * 128
                    ps = ppool.tile([128, F], mybir.dt.float32)
                    nc.tensor.matmul(ps[:], x_t[:, o:o + 128], w_t[:, 0:F],
                                     start=True, stop=False)
                    nc.tensor.matmul(ps[:], x_t[:, XB * 128 + o:XB * 128 + o + 128],
                                     w_t[:, F:2 * F], start=False, stop=True)
                    nc.vector.tensor_add(h_t[:, o:o + F], ps[:], b_t[:])
                nc.sync.dma_start(out=h_p[:, g0 * F:g1 * F], in_=h_t[:, 0:nwin * F])

    nc.finalize()
    return nc


def _build_launch2(CC):
    SLOT = CC * 128                  # edge slots per cell
    TOK = NCELL * SLOT               # tokens per core
    BTOK = NW * SLOT                 # tokens per bucket stream
    GCALL = 1024                     # tokens per gather call; one call's
                                     # descriptors must fit the SWDGE ring
                                     # (HW-probed: 1024 ok, 1536+ crashes)
    WG = 4                           # windows per output-stage DMA batch

    # per-bucket gather call token ranges [start, end)
    calls = [(s, min(s + GCALL, BTOK)) for s in range(0, BTOK, GCALL)]

    nc = bacc.Bacc(None, target_bir_lowering=False)
    h_ag = nc.declare_dram_parameter("h_ag", [N_NODES, F], mybir.dt.bfloat16, isOutput=False)
    idx_p = nc.declare_dram_parameter("tok_idx", [128, TOK // 16], mybir.dt.int16, isOutput=False)
    rr_p = nc.declare_dram_parameter("row_rel", [128, TOK // 128], mybir.dt.float32, isOutput=False)
    val_p = nc.declare_dram_parameter("val", [128, TOK // 128], mybir.dt.float32, isOutput=False)
    iota_p = nc.declare_dram_parameter("iota", [128, 128], mybir.dt.bfloat16, isOutput=False)
    # out in tiled layout [128, NW*F]; host un-tiles.
    out_p = nc.declare_dram_parameter("out", [128, NW * F], mybir.dt.float32, isOutput=True)

    # window groups per gather call: [0..3], [4..7], ..., tail [96..97]
    wgroups = [(s, min(s + WG, NW)) for s in range(0, NW, WG)]

    with tile.TileContext(nc) as tc:
        with (
            tc.tile_pool(name="const", bufs=1) as cpool,
            tc.tile_pool(name="meta", bufs=1) as mpool,
            tc.tile_pool(name="msgs", bufs=3) as gpool,
            tc.tile_pool(name="s", bufs=6) as spool,
            tc.tile_pool(name="ostage", bufs=4) as opool,
            tc.tile_pool(name="psum", bufs=4, space="PSUM") as ppool,
        ):
            iota_t = cpool.tile([128, 128], mybir.dt.bfloat16)
            nc.sync.dma_start(out=iota_t[:], in_=iota_p[:])
            idx_t = mpool.tile([128, TOK // 16], mybir.dt.int16)
            nc.sync.dma_start(out=idx_t[:], in_=idx_p[:])
            rr_t = mpool.tile([128, TOK // 128], mybir.dt.float32)
            nc.sync.dma_start(out=rr_t[:], in_=rr_p[:])
            val_t = mpool.tile([128, TOK // 128], mybir.dt.float32)
            nc.sync.dma_start(out=val_t[:], in_=val_p[:])

            # msgs tiles, keyed by (bucket, call index)
            msgs = {}

            def issue_gather(b, ci):
                c0, c1 = calls[ci]
                ntok = c1 - c0
                t0 = b * BTOK + c0           # token offset in core stream
                mt = gpool.tile([128, GCALL], mybir.dt.bfloat16, tag=f"msgs{b}")
                nc.gpsimd.dma_gather(
                    mt[:, 0:ntok].rearrange("p (c e) -> p c e", e=F),
                    h_ag[b * BUCKET:(b + 1) * BUCKET, :],
                    idx_t[:, t0 // 16:(t0 + ntok) // 16],
                    ntok,
                    ntok,
                    F,
                )
                msgs[(b, ci)] = mt

            issued = 0

            for gi in range(len(wgroups)):
                w0, w1 = wgroups[gi]
                # issue any gather call needed by windows up to w1
                need = min(len(calls), (w1 * SLOT + GCALL - 1) // GCALL)
                while issued < need:
                    for b in range(NB):
                        issue_gather(b, issued)
                    issued += 1
                ot = opool.tile([128, WG * F], mybir.dt.float32)
                for w in range(w0, w1):
                    ps = ppool.tile([128, F], mybir.dt.float32)
                    nq = NB * CC
                    q = 0
                    for b in range(NB):
                        for k in range(CC):
                            col = (b * NW + w) * CC + k       # chunk column in rr/val
                            o = w * SLOT + k * 128            # bucket-stream offset
                            ci = o // GCALL
                            mt = msgs[(b, ci)]
                            ccol = (o - calls[ci][0]) // 128  # chunk within gather tile
                            s_t = spool.tile([128, 128], mybir.dt.bfloat16)
                            nc.vector.tensor_scalar(
                                s_t[:], iota_t[:],
                                rr_t[:, col:col + 1], val_t[:, col:col + 1],
                                mybir.AluOpType.is_equal, mybir.AluOpType.mult,
                            )
                            nc.tensor.matmul(
                                ps[:], s_t[:], mt[:, ccol * F:(ccol + 1) * F],
                                start=(q == 0), stop=(q == nq - 1),
                            )
                            q += 1
                    nc.vector.tensor_copy(ot[:, (w - w0) * F:(w - w0 + 1) * F], ps[:])
                nc.sync.dma_start(out=out_p[:, w0 * F:w1 * F],
                                  in_=ot[:, 0:(w1 - w0) * F])

    nc.finalize()
    return nc


def _prep_edges(edge_row, edge_col, edge_val):
    """Pack edges into the fixed per-(core,bucket,window) cell layout.

    Returns (CC, tok_idx, row_rel, val) where the arrays are the global
    token streams of all cores concatenated (core-major, then bucket,
    then window, then slot)."""
    E = edge_row.shape[0]
    core = (edge_row // R).astype(np.int64)
    r_loc = (edge_row - core * R).astype(np.int64)
    w = r_loc >> 7
    row_rel = (r_loc & 127).astype(np.float32)
    b = (edge_col // BUCKET).astype(np.int64)
    idx16 = (edge_col - b * BUCKET).astype(np.int16)

    cid = (core * NB + b) * NW + w           # global cell id, core-major
    # sort within cell by col so gather addresses ascend (HBM locality)
    order = np.lexsort((idx16, cid))
    scid = cid[order]
    counts = np.bincount(cid, minlength=N_CORES * NCELL)
    CC = int((counts.max() + 127) // 128)
    SLOT = CC * 128
    starts = np.zeros(N_CORES * NCELL, np.int64)
    np.cumsum(counts[:-1], out=starts[1:])
    rank = np.arange(E, dtype=np.int64) - starts[scid]
    pos = scid * SLOT + rank

    TOKALL = N_CORES * NCELL * SLOT
    tok_idx = np.zeros(TOKALL, np.int16)
    rr = np.zeros(TOKALL, np.float32)
    vv = np.zeros(TOKALL, np.float32)
    tok_idx[pos] = idx16[order]
    rr[pos] = row_rel[order]
    vv[pos] = edge_val[order].astype(BF16).astype(np.float32)
    return CC, tok_idx, rr, vv


def kernel(X, edge_row, edge_col, edge_val, W, b):
    X = np.asarray(X, dtype=np.float32)
    W = np.asarray(W, dtype=np.float32)
    b = np.asarray(b, dtype=np.float32)
    edge_row = np.asarray(edge_row).astype(np.int64)
    edge_col = np.asarray(edge_col).astype(np.int64)
    edge_val = np.asarray(edge_val, dtype=np.float32)
    cores = list(range(N_CORES))

    # ---- launch 1: H = X @ W^T + b, node-sharded ----
    if "l1" not in _cache:
        _cache["l1"] = _build_launch1()
    nc1 = _cache["l1"]

    wt = np.ascontiguousarray(W.T).astype(BF16)           # [256, 128]
    bb = np.tile(b[None, :], (128, 1)).astype(np.float32)  # [128, 128]
    in_maps1 = []
    for c in cores:
        xt = np.zeros((IN_DIM, RPAD), BF16)
        xt[:, :R] = X[c * R:(c + 1) * R].T.astype(BF16)
        in_maps1.append({"xt": xt, "wt": wt, "bb": bb})
    res1 = run_bass_kernel_spmd(nc1, in_maps1, core_ids=cores)
    # un-tile [128, NW*F] -> [RPAD, F], drop pad rows, concat shards
    H_full = np.concatenate([
        res1.results[c]["h"].reshape(128, NW, F).transpose(1, 0, 2).reshape(RPAD, F)[:R]
        for c in cores
    ], axis=0)

    # ---- host: pack edges ----
    CC, tok_idx, rr, vv = _prep_edges(edge_row, edge_col, edge_val)
    SLOT = CC * 128
    TOK = NCELL * SLOT

    key = ("l2", CC)
    if key not in _cache:
        _cache[key] = _build_launch2(CC)
    nc2 = _cache[key]

    iota = np.tile(np.arange(128, dtype=np.float32).astype(BF16), (128, 1))
    in_maps2 = []
    for c in cores:
        sl = slice(c * TOK, (c + 1) * TOK)
        ti = np.tile(tok_idx[sl].reshape(-1, 16).T, (8, 1)).copy()
        in_maps2.append({
            "h_ag": H_full,
            "tok_idx": ti,
            "row_rel": np.ascontiguousarray(rr[sl].reshape(-1, 128).T),
            "val": np.ascontiguousarray(vv[sl].reshape(-1, 128).T),
            "iota": iota,
        })
    res2 = run_bass_kernel_spmd(nc2, in_maps2, core_ids=cores)
    out = np.concatenate([
        res2.results[c]["out"].reshape(128, NW, F).transpose(1, 0, 2).reshape(RPAD, F)[:R]
        for c in cores
    ], axis=0)
    return out
